# revision 28
# baseline (speedup 1.0000x reference)
"""Causal MHA (B=2, S=2048, D=1024, H=16) sharded over 8 NeuronCores.

Sharding: head-parallel. Core c owns heads {2c, 2c+1} for both batches:
Wq/Wk/Wv split by head rows (column-parallel), Wo split by head columns
(row-parallel); per-core fp16 partial outputs are summed on the host.

v3 pipeline:
  x arrives pre-transposed from host as xT[b, d, s] -> plain strided DMA
  (sync queue; const weights on the scalar queue in parallel).
  Per 512-token chunk c: project Q^T/K^T (W-stationary fp16, N=512),
  RoPE via P2 matmul + cos/sin (muls on gpsimd/vector); project V^T the
  same way, then PE-transpose 128x128 tiles into vn[s, h, t, 80] with
  col 64 = ones (softmax denominator row).  Attention for q-chunk c
  follows immediately (kv tiles 0..4c+3): both heads' S^T matmuls are
  issued adjacently (row-group packed, K=64 -> concurrent), one merged
  exp over both heads, causal mask via gpsimd affine_select, PV
  accumulates [65, 512] psum (row 64 = denominator).  The epilogue
  (reciprocal_approx_fast straight from PSUM, gpsimd partition
  broadcast, normalize into stackT) and the out-projection are
  pipelined ONE CHUNK BehinD so they never stall the tensor queue.
"""

import sys

import numpy as np

sys.path.insert(0, "/opt/trn_rl_repo")

B, S, D, H = 2, 2048, 1024, 16
DK = D // H            # 64
NCORES = 8
HPC = H // NCORES      # 2 heads per core
DKH = HPC * DK         # 128 local head-dim
THETA = 10000.0
SCALE = 1.0 / float(np.sqrt(DK))

W = 512                # q-chunk width
NCH = S // W           # 4 chunks
NT = S // 128          # 16 kv tiles of 128
VROW = 80              # per-head row block in vn (16-aligned, col 64 = ones)


def _rope_tables():
    pos = np.arange(S, dtype=np.float64)
    dim = np.arange(0, DK, 2, dtype=np.float64)
    inv_freq = 1.0 / THETA ** (dim / DK)
    angle = pos[None, :] * inv_freq[:, None]        # [DK/2, S]
    angle = np.repeat(angle, 2, axis=0)             # [DK, S] interleaved rows
    cos1, sin1 = np.cos(angle), np.sin(angle)
    cosT = np.concatenate([cos1, cos1], axis=0).astype(np.float16)  # [128, S]
    sinT = np.concatenate([sin1, sin1], axis=0).astype(np.float16)
    return cosT, sinT


def _p2t():
    # pair rotation: out[2i] = -in[2i+1]; out[2i+1] = +in[2i], per 64-row head.
    # matmul computes lhsT.T @ rhs, so pass P2^T.
    p = np.zeros((DK, DK), dtype=np.float32)
    for i in range(DK // 2):
        p[2 * i, 2 * i + 1] = -1.0
        p[2 * i + 1, 2 * i] = 1.0
    p2 = np.zeros((DKH, DKH), dtype=np.float32)
    p2[:DK, :DK] = p
    p2[DK:, DK:] = p
    return np.ascontiguousarray(p2.T).astype(np.float16)


CONST_COLS = 8320  # wq|wk|wv 3x[128,8,128] + wo [128,1024] + cos|sin [128,2048] + p2 [128,128]


def make_in_maps(x, Wq, Wk, Wv, Wo):
    cosT, sinT = _rope_tables()
    p2t = _p2t()
    x16 = np.asarray(x, dtype=np.float32).astype(np.float16)
    xt = np.ascontiguousarray(x16.transpose(0, 2, 1))  # [B, D, S]
    Wq, Wk, Wv, Wo = (np.asarray(w, dtype=np.float32) for w in (Wq, Wk, Wv, Wo))
    in_maps = []
    for c in range(NCORES):
        rows = slice(c * DKH, (c + 1) * DKH)
        cb = np.zeros((128, CONST_COLS), dtype=np.float16)
        # w*T [D, DKH] -> [128, 8, 128]: cb[p, 128j+m] = W^T[128j+p, m]
        for off, Wm in ((0, Wq), (1024, Wk), (2048, Wv)):
            wt = Wm[rows, :].T.astype(np.float16)  # [D, DKH]
            cb[:, off : off + 1024] = (
                wt.reshape(8, 128, DKH).transpose(1, 0, 2).reshape(128, 1024)
            )
        cb[:, 3072:4096] = Wo[:, rows].T.astype(np.float16)  # [DKH, D]
        cb[:, 4096:6144] = cosT
        cb[:, 6144:8192] = sinT
        cb[:, 8192:8320] = p2t
        in_maps.append({"xt": xt, "cb": np.ascontiguousarray(cb)})
    return in_maps


def _build_nc():
    from contextlib import ExitStack

    import concourse.tile as tile
    from concourse import bacc, mybir

    fp16 = mybir.dt.float16
    f32 = mybir.dt.float32
    EXP = mybir.ActivationFunctionType.Exp
    IS_GE = mybir.AluOpType.is_ge
    MULT = mybir.AluOpType.mult

    nc = bacc.Bacc(
        "TRN2", target_bir_lowering=False, debug=False, num_devices=NCORES
    )
    xt_d = nc.dram_tensor("xt", [B, D, S], fp16, kind="ExternalInput")
    cb_d = nc.dram_tensor("cb", [128, CONST_COLS], fp16, kind="ExternalInput")
    out_d = nc.dram_tensor("out", [B, S, D], fp16, kind="ExternalOutput")

    with tile.TileContext(nc) as tc, ExitStack() as ctx:
        consts = ctx.enter_context(tc.tile_pool(name="consts", bufs=1))
        xtp = ctx.enter_context(tc.tile_pool(name="xt", bufs=8))
        qkraw = ctx.enter_context(tc.tile_pool(name="qkraw", bufs=4))
        qkrope = ctx.enter_context(tc.tile_pool(name="qkrope", bufs=4))
        ropetmp = ctx.enter_context(tc.tile_pool(name="ropetmp", bufs=4))
        vtsb = ctx.enter_context(tc.tile_pool(name="vtsb", bufs=2))
        vnp = ctx.enter_context(tc.tile_pool(name="vn", bufs=2))
        ptp = ctx.enter_context(tc.tile_pool(name="pt", bufs=3))
        stackp = ctx.enter_context(tc.tile_pool(name="stack", bufs=2))
        outp = ctx.enter_context(tc.tile_pool(name="outsb", bufs=3))
        epip = ctx.enter_context(tc.tile_pool(name="epi", bufs=4))
        bcp = ctx.enter_context(tc.tile_pool(name="bc", bufs=4))

        # PSUM: 'big' ring 2x[128,1024]f32 (4 banks) shared by projections /
        # rope / V / V-transpose / scores / out-proj; 'acc' ring 4x[65,512]
        # (4 banks) for the PV accumulators (2 live chunks).
        psb = ctx.enter_context(tc.tile_pool(name="psb", bufs=2, space="PSUM"))
        psa = ctx.enter_context(tc.tile_pool(name="psa", bufs=4, space="PSUM"))

        # ---- constants: two packed DMAs on the scalar queue (xt on sync):
        # the QKV weights first (they gate the first projection), the rest
        # (wo/cos/sin/p2, needed a few us later) second.
        cb_sb = consts.tile([128, CONST_COLS], fp16, tag="cb")
        nc.scalar.dma_start(cb_sb[:, 0:3072], cb_d[:, 0:3072])
        nc.scalar.dma_start(cb_sb[:, 3072:], cb_d[:, 3072:])
        WQ0, WK0, WV0 = 0, 1024, 2048
        wo_sb = cb_sb[:, 3072:4096]
        cos_sb = cb_sb[:, 4096:6144]
        sin_sb = cb_sb[:, 6144:8192]
        p2_sb = cb_sb[:, 8192:8320]
        ident = consts.tile([128, 128], fp16, tag="ident")
        nc.vector.memset(ident, 1.0)
        nc.gpsimd.affine_select(
            out=ident,
            in_=ident,
            pattern=[[1, 128]],
            compare_op=mybir.AluOpType.is_equal,
            fill=0.0,
            base=0,
            channel_multiplier=-1,
        )

        pending_epis = []  # deferred epilogue closures (1-chunk lag)
        pending_outs = []  # deferred out-projection closures (1-chunk lag)

        for b in range(B):
            # x^T chunks [d-part, j, s]: xt[p, j, r] = xT[b, 128j+p, 512c+r]
            xts = []
            for cch in range(4):
                xt = xtp.tile([128, 8, W], fp16, tag="xt")
                src = xt_d[b].rearrange("(j p) s -> p j s", p=128)[
                    :, :, W * cch : W * (cch + 1)
                ]
                nc.sync.dma_start(xt, src)
                xts.append(xt)

            qtr = qkrope.tile([DKH, S], fp16, tag="qkrope")
            ktr = qkrope.tile([DKH, S], fp16, tag="qkrope")
            vn = vnp.tile([128, HPC, NT, VROW], fp16, tag="vn")
            nc.vector.memset(vn[:, :, :, 64:65], 1.0)
            stackT = stackp.tile([DKH, S], fp16, tag="stack")

            def emit_proj(cch):
                sl = slice(W * cch, W * (cch + 1))

                # all three projection matmul groups first; rope and
                # V-transpose matmuls follow AFTER, so the DVE casts they
                # depend on are long done when the tensor queue reaches them.
                def proj_mms(wbase, dst_ps):
                    for j in range(8):
                        nc.tensor.matmul(
                            dst_ps,
                            cb_sb[:, wbase + 128 * j : wbase + 128 * (j + 1)],
                            xts[cch][:, j, :],
                            start=(j == 0),
                            stop=(j == 7),
                        )

                ps_q = psb.tile([128, W], f32, tag="big")
                proj_mms(WQ0, ps_q)
                raw_q = qkraw.tile([DKH, W], fp16, tag="qkraw")
                nc.vector.tensor_copy(raw_q, ps_q)
                ps_k = psb.tile([128, W], f32, tag="big")
                proj_mms(WK0, ps_k)
                raw_k = qkraw.tile([DKH, W], fp16, tag="qkraw")
                nc.vector.tensor_copy(raw_k, ps_k)
                vps = psb.tile([128, W], f32, tag="big")
                proj_mms(WV0, vps)
                vt_sb = vtsb.tile([DKH, W], fp16, tag="vtsb")
                nc.vector.tensor_copy(vt_sb, vps)

                def rope(raw, dst):
                    ps2 = psb.tile([128, W], f32, tag="big")
                    nc.tensor.matmul(ps2, p2_sb, raw, start=True, stop=True)
                    t1 = ropetmp.tile([DKH, W], fp16, tag="ropetmp")
                    nc.vector.tensor_mul(t1, raw, cos_sb[:, sl])
                    t2 = ropetmp.tile([DKH, W], fp16, tag="ropetmp")
                    nc.vector.tensor_mul(t2, ps2, sin_sb[:, sl])
                    nc.vector.tensor_add(dst[:, sl], t1, t2)

                rope(raw_q, qtr)
                rope(raw_k, ktr)

                tp = psb.tile([128, 4, 128], fp16, tag="big")
                for k in range(4):
                    nc.tensor.transpose(
                        tp[:, k, :], vt_sb[:, 128 * k : 128 * (k + 1)], ident
                    )
                # vn[p, h, 4c+k, 0:64] = tp[p, k, 64h:64h+64]
                for h in range(HPC):
                    nc.vector.tensor_copy(
                        vn[:, h, 4 * cch : 4 * cch + 4, 0:64],
                        tp[:, :, 64 * h : 64 * (h + 1)],
                    )

            for cch in range(NCH):
                sl = slice(W * cch, W * (cch + 1))

                # projections run one chunk ahead: their rope chain (PSUM ->
                # DVE cast -> P2 matmul -> DVE muls) hides under this chunk's
                # attention instead of stalling the next chunk's scores.
                if cch == 0:
                    emit_proj(0)
                if cch + 1 < NCH:
                    emit_proj(cch + 1)

                # ---- deferred epilogue of the previous chunk (DVE/gpsimd) --
                if pending_epis:
                    pending_epis.pop(0)()

                # ---- attention for q-chunk cch (kv tiles 0..4c+3) ----
                at0 = psa.tile([65, W], f32, tag="acc")
                at1 = psa.tile([65, W], f32, tag="acc")
                ats = (at0, at1)
                n_kt = 4 * cch + 4

                def emit_pv(t, pt, qs, ats=ats, n_kt=n_kt, vn=vn):
                    for h in range(HPC):
                        nc.tensor.matmul(
                            ats[h][:, qs:W],
                            vn[:, h, t, 0:65],
                            pt[:, h, qs:W],
                            start=(t == 0),
                            stop=(t == n_kt - 1),
                        )

                pending = None  # software-pipeline PV one kv-tile back
                for t in range(n_kt):
                    qs = max(128 * t - W * cch, 0)
                    sc = psb.tile([128, HPC, W], f32, tag="big")
                    for h in range(HPC):
                        hsl = slice(DK * h, DK * (h + 1))
                        nc.tensor.matmul(
                            sc[:, h, qs:W],
                            ktr[hsl, 128 * t : 128 * (t + 1)],
                            qtr[hsl, W * cch + qs : W * (cch + 1)],
                            start=True,
                            stop=True,
                        )
                    pt = ptp.tile([128, HPC, W], fp16, tag="pt")
                    nc.scalar.activation(
                        pt[:, :, qs:W], sc[:, :, qs:W], EXP, scale=SCALE
                    )
                    if t >= 4 * cch:  # diagonal tile: causal mask per head
                        for h in range(HPC):
                            nc.gpsimd.affine_select(
                                out=pt[:, h, qs : qs + 128],
                                in_=pt[:, h, qs : qs + 128],
                                pattern=[[1, 128]],
                                compare_op=IS_GE,
                                fill=0.0,
                                base=0,
                                channel_multiplier=-1,
                            )
                    if pending is not None:
                        emit_pv(*pending)
                    pending = (t, pt, qs)
                emit_pv(*pending)

                # ---- epilogue + out-proj, deferred one chunk ----
                def make_epi(b=b, cch=cch, ats=ats, stackT=stackT, sl=sl):
                    def epi():
                        denp = epip.tile([1, HPC, W], f32, tag="denp")
                        for h in range(HPC):
                            nc.vector.tensor_copy(denp[0:1, h, :], ats[h][64:65, :])
                        denr = epip.tile([1, HPC, W], f32, tag="denr")
                        nc.vector.reciprocal_approx_fast(denr, denp)
                        denr16 = epip.tile([1, HPC, W], fp16, tag="denr16")
                        nc.vector.tensor_copy(denr16, denr)
                        for h in range(HPC):
                            bc = bcp.tile([DK, W], fp16, tag="bc")
                            nc.gpsimd.partition_broadcast(bc, denr16[0:1, h, :])
                            nc.vector.tensor_tensor(
                                stackT[DK * h : DK * (h + 1), sl],
                                ats[h][0:64, :],
                                bc,
                                op=MULT,
                            )

                    def outproj():
                        for k in range(4):
                            qt_i = 4 * cch + k
                            ant = stackT[:, 128 * qt_i : 128 * (qt_i + 1)]
                            po = psb.tile([128, D], f32, tag="big")
                            for oc in range(2):
                                nc.tensor.matmul(
                                    po[:, 512 * oc : 512 * (oc + 1)],
                                    ant,
                                    wo_sb[:, 512 * oc : 512 * (oc + 1)],
                                    start=True,
                                    stop=True,
                                )
                            osb = outp.tile([128, D], fp16, tag="osb")
                            if k % 2 == 0:
                                nc.vector.tensor_copy(osb, po)
                            else:
                                nc.scalar.copy(osb, po)
                            nc.sync.dma_start(
                                out_d[b, 128 * qt_i : 128 * (qt_i + 1), :], osb
                            )

                    return epi, outproj

                epi, outproj = make_epi()
                pending_epis.append(epi)
                pending_outs.append(outproj)

                # previous chunk's out-proj lands behind this chunk's
                # attention in the tensor queue: its epilogue is long done.
                if len(pending_outs) > 1:
                    pending_outs.pop(0)()

        while pending_epis:
            pending_epis.pop(0)()
        while pending_outs:
            pending_outs.pop(0)()

    nc.compile()
    return nc


_NC_CACHE = None


def kernel(x, Wq, Wk, Wv, Wo):
    global _NC_CACHE
    from concourse.bass_utils import run_bass_kernel_spmd

    if _NC_CACHE is None:
        _NC_CACHE = _build_nc()
    nc = _NC_CACHE

    in_maps = make_in_maps(x, Wq, Wk, Wv, Wo)
    res = run_bass_kernel_spmd(nc, in_maps, core_ids=list(range(NCORES)))
    out = np.zeros((B, S, D), dtype=np.float32)
    for r in res.results:
        out += r["out"].astype(np.float32)
    return out


# revision 29
# speedup vs baseline: 1.0106x; 1.0106x over previous
"""Causal MHA (B=2, S=2048, D=1024, H=16) sharded over 8 NeuronCores.

Sharding: head-parallel. Core c owns heads {2c, 2c+1} for both batches:
Wq/Wk/Wv split by head rows (column-parallel), Wo split by head columns
(row-parallel); per-core fp16 partial outputs are summed on the host.

v3 pipeline:
  x arrives pre-transposed from host as xT[b, d, s] -> plain strided DMA
  (sync queue; const weights on the scalar queue in parallel).
  Per 512-token chunk c: project Q^T/K^T (W-stationary fp16, N=512),
  RoPE via P2 matmul + cos/sin (muls on gpsimd/vector); project V^T the
  same way, then PE-transpose 128x128 tiles into vn[s, h, t, 80] with
  col 64 = ones (softmax denominator row).  Attention for q-chunk c
  follows immediately (kv tiles 0..4c+3): both heads' S^T matmuls are
  issued adjacently (row-group packed, K=64 -> concurrent), one merged
  exp over both heads, causal mask via gpsimd affine_select, PV
  accumulates [65, 512] psum (row 64 = denominator).  The epilogue
  (reciprocal_approx_fast straight from PSUM, gpsimd partition
  broadcast, normalize into stackT) and the out-projection are
  pipelined ONE CHUNK BehinD so they never stall the tensor queue.
"""

import sys

import numpy as np

sys.path.insert(0, "/opt/trn_rl_repo")

B, S, D, H = 2, 2048, 1024, 16
DK = D // H            # 64
NCORES = 8
HPC = H // NCORES      # 2 heads per core
DKH = HPC * DK         # 128 local head-dim
THETA = 10000.0
SCALE = 1.0 / float(np.sqrt(DK))

W = 512                # q-chunk width
NCH = S // W           # 4 chunks
NT = S // 128          # 16 kv tiles of 128
VROW = 80              # per-head row block in vn (16-aligned, col 64 = ones)


def _rope_tables():
    pos = np.arange(S, dtype=np.float64)
    dim = np.arange(0, DK, 2, dtype=np.float64)
    inv_freq = 1.0 / THETA ** (dim / DK)
    angle = pos[None, :] * inv_freq[:, None]        # [DK/2, S]
    angle = np.repeat(angle, 2, axis=0)             # [DK, S] interleaved rows
    cos1, sin1 = np.cos(angle), np.sin(angle)
    cosT = np.concatenate([cos1, cos1], axis=0).astype(np.float16)  # [128, S]
    sinT = np.concatenate([sin1, sin1], axis=0).astype(np.float16)
    return cosT, sinT


def _p2t():
    # pair rotation: out[2i] = -in[2i+1]; out[2i+1] = +in[2i], per 64-row head.
    # matmul computes lhsT.T @ rhs, so pass P2^T.
    p = np.zeros((DK, DK), dtype=np.float32)
    for i in range(DK // 2):
        p[2 * i, 2 * i + 1] = -1.0
        p[2 * i + 1, 2 * i] = 1.0
    p2 = np.zeros((DKH, DKH), dtype=np.float32)
    p2[:DK, :DK] = p
    p2[DK:, DK:] = p
    return np.ascontiguousarray(p2.T).astype(np.float16)


CONST_COLS = 8320  # wq|wk|wv 3x[128,8,128] + wo [128,1024] + cos|sin [128,2048] + p2 [128,128]


def make_in_maps(x, Wq, Wk, Wv, Wo):
    cosT, sinT = _rope_tables()
    p2t = _p2t()
    x16 = np.asarray(x, dtype=np.float32).astype(np.float16)
    xt = np.ascontiguousarray(x16.transpose(0, 2, 1))  # [B, D, S]
    Wq, Wk, Wv, Wo = (np.asarray(w, dtype=np.float32) for w in (Wq, Wk, Wv, Wo))
    in_maps = []
    for c in range(NCORES):
        rows = slice(c * DKH, (c + 1) * DKH)
        cb = np.zeros((128, CONST_COLS), dtype=np.float16)
        # w*T [D, DKH] -> [128, 8, 128]: cb[p, 128j+m] = W^T[128j+p, m]
        for off, Wm in ((0, Wq), (1024, Wk), (2048, Wv)):
            wt = Wm[rows, :].T.astype(np.float16)  # [D, DKH]
            cb[:, off : off + 1024] = (
                wt.reshape(8, 128, DKH).transpose(1, 0, 2).reshape(128, 1024)
            )
        cb[:, 3072:4096] = Wo[:, rows].T.astype(np.float16)  # [DKH, D]
        cb[:, 4096:6144] = cosT
        cb[:, 6144:8192] = sinT
        cb[:, 8192:8320] = p2t
        in_maps.append({"xt": xt, "cb": np.ascontiguousarray(cb)})
    return in_maps


def _build_nc():
    from contextlib import ExitStack

    import concourse.tile as tile
    from concourse import bacc, mybir

    fp16 = mybir.dt.float16
    f32 = mybir.dt.float32
    EXP = mybir.ActivationFunctionType.Exp
    IS_GE = mybir.AluOpType.is_ge
    MULT = mybir.AluOpType.mult

    nc = bacc.Bacc(
        "TRN2", target_bir_lowering=False, debug=False, num_devices=NCORES
    )
    xt_d = nc.dram_tensor("xt", [B, D, S], fp16, kind="ExternalInput")
    cb_d = nc.dram_tensor("cb", [128, CONST_COLS], fp16, kind="ExternalInput")
    out_d = nc.dram_tensor("out", [B, S, D], fp16, kind="ExternalOutput")

    with tile.TileContext(nc) as tc, ExitStack() as ctx:
        consts = ctx.enter_context(tc.tile_pool(name="consts", bufs=1))
        xtp = ctx.enter_context(tc.tile_pool(name="xt", bufs=8))
        qkraw = ctx.enter_context(tc.tile_pool(name="qkraw", bufs=4))
        qkrope = ctx.enter_context(tc.tile_pool(name="qkrope", bufs=4))
        ropetmp = ctx.enter_context(tc.tile_pool(name="ropetmp", bufs=4))
        vtsb = ctx.enter_context(tc.tile_pool(name="vtsb", bufs=2))
        vnp = ctx.enter_context(tc.tile_pool(name="vn", bufs=2))
        ptp = ctx.enter_context(tc.tile_pool(name="pt", bufs=3))
        stackp = ctx.enter_context(tc.tile_pool(name="stack", bufs=2))
        outp = ctx.enter_context(tc.tile_pool(name="outsb", bufs=3))
        epip = ctx.enter_context(tc.tile_pool(name="epi", bufs=4))
        bcp = ctx.enter_context(tc.tile_pool(name="bc", bufs=4))

        # PSUM: 'big' ring 2x[128,1024]f32 (4 banks) shared by projections /
        # rope / V / V-transpose / scores / out-proj; 'acc' ring 4x[65,512]
        # (4 banks) for the PV accumulators (2 live chunks).
        psb = ctx.enter_context(tc.tile_pool(name="psb", bufs=2, space="PSUM"))
        psa = ctx.enter_context(tc.tile_pool(name="psa", bufs=4, space="PSUM"))

        # ---- constants: two packed DMAs on the scalar queue (xt on sync):
        # the QKV weights first (they gate the first projection), the rest
        # (wo/cos/sin/p2, needed a few us later) second.
        cb_sb = consts.tile([128, CONST_COLS], fp16, tag="cb")
        nc.scalar.dma_start(cb_sb[:, 0:3072], cb_d[:, 0:3072])
        nc.scalar.dma_start(cb_sb[:, 3072:], cb_d[:, 3072:])
        WQ0, WK0, WV0 = 0, 1024, 2048
        wo_sb = cb_sb[:, 3072:4096]
        cos_sb = cb_sb[:, 4096:6144]
        sin_sb = cb_sb[:, 6144:8192]
        p2_sb = cb_sb[:, 8192:8320]
        ident = consts.tile([128, 128], fp16, tag="ident")
        nc.vector.memset(ident, 1.0)
        nc.gpsimd.affine_select(
            out=ident,
            in_=ident,
            pattern=[[1, 128]],
            compare_op=mybir.AluOpType.is_equal,
            fill=0.0,
            base=0,
            channel_multiplier=-1,
        )

        pending_epis = []  # deferred epilogue closures (1-chunk lag)
        pending_outs = []  # deferred out-projection closures (1-chunk lag)

        for b in range(B):
            # x^T chunks [d-part, j, s]: xt[p, j, r] = xT[b, 128j+p, 512c+r]
            xts = []
            for cch in range(4):
                xt = xtp.tile([128, 8, W], fp16, tag="xt")
                src = xt_d[b].rearrange("(j p) s -> p j s", p=128)[
                    :, :, W * cch : W * (cch + 1)
                ]
                nc.sync.dma_start(xt, src)
                xts.append(xt)

            qtr = qkrope.tile([DKH, S], fp16, tag="qkrope")
            ktr = qkrope.tile([DKH, S], fp16, tag="qkrope")
            vn = vnp.tile([128, HPC, NT, VROW], fp16, tag="vn")
            nc.vector.memset(vn[:, :, :, 64:65], 1.0)
            stackT = stackp.tile([DKH, S], fp16, tag="stack")

            def emit_proj(cch):
                sl = slice(W * cch, W * (cch + 1))

                # all three projection matmul groups first; rope and
                # V-transpose matmuls follow AFTER, so the DVE casts they
                # depend on are long done when the tensor queue reaches them.
                def proj_mms(wbase, dst_ps):
                    for j in range(8):
                        nc.tensor.matmul(
                            dst_ps,
                            cb_sb[:, wbase + 128 * j : wbase + 128 * (j + 1)],
                            xts[cch][:, j, :],
                            start=(j == 0),
                            stop=(j == 7),
                        )

                ps_q = psb.tile([128, W], f32, tag="big")
                proj_mms(WQ0, ps_q)
                raw_q = qkraw.tile([DKH, W], fp16, tag="qkraw")
                nc.scalar.copy(raw_q, ps_q)
                ps_k = psb.tile([128, W], f32, tag="big")
                proj_mms(WK0, ps_k)
                raw_k = qkraw.tile([DKH, W], fp16, tag="qkraw")
                nc.scalar.copy(raw_k, ps_k)
                vps = psb.tile([128, W], f32, tag="big")
                proj_mms(WV0, vps)
                vt_sb = vtsb.tile([DKH, W], fp16, tag="vtsb")
                nc.vector.tensor_copy(vt_sb, vps)

                def rope(raw, dst):
                    ps2 = psb.tile([128, W], f32, tag="big")
                    nc.tensor.matmul(ps2, p2_sb, raw, start=True, stop=True)
                    t1 = ropetmp.tile([DKH, W], fp16, tag="ropetmp")
                    nc.vector.tensor_mul(t1, raw, cos_sb[:, sl])
                    t2 = ropetmp.tile([DKH, W], fp16, tag="ropetmp")
                    nc.vector.tensor_mul(t2, ps2, sin_sb[:, sl])
                    nc.vector.tensor_add(dst[:, sl], t1, t2)

                rope(raw_q, qtr)
                rope(raw_k, ktr)

                tp = psb.tile([128, 4, 128], fp16, tag="big")
                for k in range(4):
                    nc.tensor.transpose(
                        tp[:, k, :], vt_sb[:, 128 * k : 128 * (k + 1)], ident
                    )
                # vn[p, h, 4c+k, 0:64] = tp[p, k, 64h:64h+64]
                for h in range(HPC):
                    nc.vector.tensor_copy(
                        vn[:, h, 4 * cch : 4 * cch + 4, 0:64],
                        tp[:, :, 64 * h : 64 * (h + 1)],
                    )

            for cch in range(NCH):
                sl = slice(W * cch, W * (cch + 1))

                # projections run one chunk ahead: their rope chain (PSUM ->
                # DVE cast -> P2 matmul -> DVE muls) hides under this chunk's
                # attention instead of stalling the next chunk's scores.
                if cch == 0:
                    emit_proj(0)
                if cch + 1 < NCH:
                    emit_proj(cch + 1)

                # ---- deferred epilogue of the previous chunk (DVE/gpsimd) --
                if pending_epis:
                    pending_epis.pop(0)()

                # ---- attention for q-chunk cch (kv tiles 0..4c+3) ----
                at0 = psa.tile([65, W], f32, tag="acc")
                at1 = psa.tile([65, W], f32, tag="acc")
                ats = (at0, at1)
                n_kt = 4 * cch + 4

                def emit_pv(t, pt, qs, ats=ats, n_kt=n_kt, vn=vn):
                    for h in range(HPC):
                        nc.tensor.matmul(
                            ats[h][:, qs:W],
                            vn[:, h, t, 0:65],
                            pt[:, h, qs:W],
                            start=(t == 0),
                            stop=(t == n_kt - 1),
                        )

                pending = None  # software-pipeline PV one kv-tile back
                for t in range(n_kt):
                    qs = max(128 * t - W * cch, 0)
                    sc = psb.tile([128, HPC, W], f32, tag="big")
                    for h in range(HPC):
                        hsl = slice(DK * h, DK * (h + 1))
                        nc.tensor.matmul(
                            sc[:, h, qs:W],
                            ktr[hsl, 128 * t : 128 * (t + 1)],
                            qtr[hsl, W * cch + qs : W * (cch + 1)],
                            start=True,
                            stop=True,
                        )
                    pt = ptp.tile([128, HPC, W], fp16, tag="pt")
                    nc.scalar.activation(
                        pt[:, :, qs:W], sc[:, :, qs:W], EXP, scale=SCALE
                    )
                    if t >= 4 * cch:  # diagonal tile: causal mask per head
                        for h in range(HPC):
                            nc.gpsimd.affine_select(
                                out=pt[:, h, qs : qs + 128],
                                in_=pt[:, h, qs : qs + 128],
                                pattern=[[1, 128]],
                                compare_op=IS_GE,
                                fill=0.0,
                                base=0,
                                channel_multiplier=-1,
                            )
                    if pending is not None:
                        emit_pv(*pending)
                    pending = (t, pt, qs)
                emit_pv(*pending)

                # ---- epilogue + out-proj, deferred one chunk ----
                def make_epi(b=b, cch=cch, ats=ats, stackT=stackT, sl=sl):
                    def epi():
                        denp = epip.tile([1, HPC, W], f32, tag="denp")
                        for h in range(HPC):
                            nc.vector.tensor_copy(denp[0:1, h, :], ats[h][64:65, :])
                        denr = epip.tile([1, HPC, W], f32, tag="denr")
                        nc.vector.reciprocal_approx_fast(denr, denp)
                        denr16 = epip.tile([1, HPC, W], fp16, tag="denr16")
                        nc.vector.tensor_copy(denr16, denr)
                        for h in range(HPC):
                            bc = bcp.tile([DK, W], fp16, tag="bc")
                            nc.gpsimd.partition_broadcast(bc, denr16[0:1, h, :])
                            nc.vector.tensor_tensor(
                                stackT[DK * h : DK * (h + 1), sl],
                                ats[h][0:64, :],
                                bc,
                                op=MULT,
                            )

                    def outproj():
                        for k in range(4):
                            qt_i = 4 * cch + k
                            ant = stackT[:, 128 * qt_i : 128 * (qt_i + 1)]
                            po = psb.tile([128, D], f32, tag="big")
                            for oc in range(2):
                                nc.tensor.matmul(
                                    po[:, 512 * oc : 512 * (oc + 1)],
                                    ant,
                                    wo_sb[:, 512 * oc : 512 * (oc + 1)],
                                    start=True,
                                    stop=True,
                                )
                            osb = outp.tile([128, D], fp16, tag="osb")
                            if k % 2 == 0:
                                nc.vector.tensor_copy(osb, po)
                            else:
                                nc.scalar.copy(osb, po)
                            nc.sync.dma_start(
                                out_d[b, 128 * qt_i : 128 * (qt_i + 1), :], osb
                            )

                    return epi, outproj

                epi, outproj = make_epi()
                pending_epis.append(epi)
                pending_outs.append(outproj)

                # previous chunk's out-proj lands behind this chunk's
                # attention in the tensor queue: its epilogue is long done.
                if len(pending_outs) > 1:
                    pending_outs.pop(0)()

        while pending_epis:
            pending_epis.pop(0)()
        while pending_outs:
            pending_outs.pop(0)()

    nc.compile()
    return nc


_NC_CACHE = None


def kernel(x, Wq, Wk, Wv, Wo):
    global _NC_CACHE
    from concourse.bass_utils import run_bass_kernel_spmd

    if _NC_CACHE is None:
        _NC_CACHE = _build_nc()
    nc = _NC_CACHE

    in_maps = make_in_maps(x, Wq, Wk, Wv, Wo)
    res = run_bass_kernel_spmd(nc, in_maps, core_ids=list(range(NCORES)))
    out = np.zeros((B, S, D), dtype=np.float32)
    for r in res.results:
        out += r["out"].astype(np.float32)
    return out
